# revision 1
# baseline (speedup 1.0000x reference)
"""TRN2 Bass kernel: batched cross-attention with padding mask.

  scores  = einsum('bdh,beh->bde', decoder_states, encoder_states)
  scores  = where(encoder_states[:,:,0]==0, -inf, scores)
  attn    = softmax(scores, -1)
  context = einsum('bde,beh->bdh', attn, encoder_states)
  returns (context, attn)

Sharding: batch dim (B=16) split across 8 NeuronCores, 2 batches per core
(pure data parallel). Each core runs the same NEFF via SPMD.

Per-core kernel design (see build_attention_nc):
  - float32r matmuls (tf32-class, 1 cycle/row at N=512) for both einsums.
  - Q^T / K^T / P^T produced by PE transposes; PSUM->SBUF copies round to
    f32r as the verifier requires.
  - The padding mask folds into the scores PSUM accumulation as a rank-1
    matmul ones[1,128].T @ bias_row[1,512], bias_row = (enc[:,:,0]==0)*-1e30.
  - Chunked softmax: each 512-wide scores chunk gets a local max/exp/sum
    as soon as its matmul chain ends (overlapping the remaining chunks on
    PE); chunks are combined with exp(m_n-M)/Z rescale multipliers.
  - The d-block loop is software-pipelined: block m's scores/softmax are
    emitted before block m-1's P^T/context matmuls.
"""

from contextlib import ExitStack

import numpy as np

import concourse.bass as bass
import concourse.mybir as mybir
import concourse.tile as tile
from concourse.masks import make_identity
from concourse.vector_clock import ScopedClock  # noqa: F401  (kept for patch parity)

F32 = mybir.dt.float32
F32R = mybir.dt.float32r
EXP = mybir.ActivationFunctionType.Exp

B, DEC, ENC, H = 16, 2048, 2048, 1024
N_CORES = 8
BPC = B // N_CORES


# ---------------------------------------------------------------------------
# Workaround: this walrus build rejects >1 sync-wait command per instruction
# ("Too many sync wait commands"). Split multi-wait instructions by hanging
# the extra waits on same-engine NOPs emitted just before them. Engine
# streams execute in order and Tile waits are monotonic, so sequential
# waiting is equivalent to the AND of all waits.
# ---------------------------------------------------------------------------
_uid = [0]


def _split_block(bb):
    insts = bb.instructions
    i = 0
    while i < len(insts):
        ins = insts[i]
        si = ins.sync_info
        waits = list(si.on_wait) if si and si.on_wait else []
        if len(waits) > 1:
            for w in waits[:-1]:
                _uid[0] += 1
                nop = mybir.InstNoOp(
                    name=f"I-waitsplit-{_uid[0]}",
                    engine=ins.engine,
                    bass_nofuse=True,
                    sync_info=mybir.SyncInfo(on_wait=[w], on_update=[]),
                )
                insts.insert(i, nop)
                i += 1
            ins.sync_info = mybir.SyncInfo(
                on_wait=[waits[-1]], on_update=si.on_update)
        i += 1


_orig_tile_exit = tile.TileContext.__exit__
_patch_done = [False]


def _patched_exit(self, *args):
    r = _orig_tile_exit(self, *args)
    for bb in self.nc.main_func.blocks:
        _split_block(bb)
    return r


def _apply_patch():
    if not _patch_done[0]:
        tile.TileContext.__exit__ = _patched_exit
        _patch_done[0] = True


class TileKernel:
    """TileContext plus an ExitStack for pools, closed in the right order."""

    def __init__(self, nc):
        self.nc = nc
        self._tc = tile.TileContext(nc)
        self._es = ExitStack()

    def __enter__(self):
        tc = self._tc.__enter__()
        tc._es = self._es
        self._es.__enter__()
        return tc

    def __exit__(self, *exc):
        self._es.__exit__(*exc)
        return self._tc.__exit__(*exc)


def build_attention_nc(B_PER_CORE=BPC, DEC=DEC, ENC=ENC, H=H, REPEAT=1,
                       mask_enabled=True, timing_mode=False):
    _apply_patch()
    nc = bass.Bass("TRN2", target_bir_lowering=False, debug=False)
    io_kind = "Internal" if timing_mode else None
    dec = nc.dram_tensor("decoder_states", [B_PER_CORE, DEC, H], F32,
                         kind=io_kind or "ExternalInput")
    enc = nc.dram_tensor("encoder_states", [B_PER_CORE, ENC, H], F32,
                         kind=io_kind or "ExternalInput")
    attn = nc.dram_tensor("attn", [B_PER_CORE, DEC, ENC], F32,
                          kind=io_kind or "ExternalOutput")
    ctx_out = nc.dram_tensor("context", [B_PER_CORE, DEC, H], F32,
                             kind=io_kind or "ExternalOutput")
    if timing_mode:
        dummy_in = nc.dram_tensor("dummy_in", [1, 16], F32,
                                  kind="ExternalInput")
        dummy_out = nc.dram_tensor("dummy_out", [1, 16], F32,
                                   kind="ExternalOutput")

    HB = H // 128
    EB = ENC // 128
    DB = DEC // 128
    NE = ENC // 512
    NH = H // 512

    with TileKernel(nc) as tc:
        es = tc._es
        const = es.enter_context(tc.tile_pool(name="const", bufs=1))
        resident = es.enter_context(tc.tile_pool(name="resident", bufs=1))
        rows = es.enter_context(tc.tile_pool(name="rows", bufs=1))
        qpool = es.enter_context(tc.tile_pool(name="qpool", bufs=2))
        qtpool = es.enter_context(tc.tile_pool(name="qtpool", bufs=2))
        ppool = es.enter_context(tc.tile_pool(name="ppool", bufs=2))
        pnpool = es.enter_context(tc.tile_pool(name="pnpool", bufs=2))
        ptpool = es.enter_context(tc.tile_pool(name="ptpool", bufs=1))
        cpool = es.enter_context(tc.tile_pool(name="cpool", bufs=1))
        stats = es.enter_context(tc.tile_pool(name="stats", bufs=3))
        ps_s = es.enter_context(tc.tile_pool(name="ps_s", bufs=1, space="PSUM"))
        ps_tr = es.enter_context(tc.tile_pool(name="ps_tr", bufs=2, space="PSUM"))
        ps_c = es.enter_context(tc.tile_pool(name="ps_c", bufs=1, space="PSUM"))

        identity = const.tile([128, 128], F32)
        make_identity(nc, identity[:])
        ones_f32 = const.tile([1, 128], F32)
        nc.vector.memset(ones_f32[:], 1.0)
        ones_col = const.tile([1, 128], F32R)
        nc.vector.tensor_copy(ones_col[:], ones_f32[:])

        e_nat = resident.tile([128, EB, H], F32R)
        kt = resident.tile([128, HB, ENC], F32R)

        def emit_scores(b, m, bias_row):
            q = qpool.tile([128, H], F32, name="q")
            nc.sync.dma_start(q[:], dec[b, m * 128:(m + 1) * 128, :])

            qt = qtpool.tile([128, HB, 128], F32R, name="qt")
            for g in range(HB // 4):
                ptile = ps_tr.tile([128, 512], F32, tag="tr", name="ptile")
                for j in range(4):
                    h = g * 4 + j
                    nc.tensor.transpose(
                        ptile[:, j * 128:(j + 1) * 128],
                        q[:, h * 128:(h + 1) * 128],
                        identity[:])
                nc.scalar.copy(
                    qt[:, g * 4:(g + 1) * 4, :].rearrange("p a b -> p (a b)"),
                    ptile[:])

            p_un = ppool.tile([128, ENC], F32, name="p_un")
            p_norm = pnpool.tile([128, ENC], F32, name="p_norm")
            negmax_c = stats.tile([128, NE], F32, tag="negmax_c",
                                  name="negmax_c")
            sum_c = stats.tile([128, NE], F32, tag="sum_c", name="sum_c")

            for n in range(NE):
                sl = slice(n * 512, (n + 1) * 512)
                s_ps = ps_s.tile([128, 512], F32, tag=f"s{n}", name=f"s_ps{n}")
                for h in range(HB):
                    nc.tensor.matmul(
                        s_ps[:], qt[:, h, :], kt[:, h, sl],
                        start=(h == 0),
                        stop=(h == HB - 1 and bias_row is None))
                if bias_row is not None:
                    nc.tensor.matmul(
                        s_ps[:], ones_col[:], bias_row[:, sl],
                        start=False, stop=True)
                nc.vector.tensor_reduce(
                    negmax_c[:, n:n + 1], s_ps[:],
                    axis=mybir.AxisListType.X,
                    op=mybir.AluOpType.max, negate=True)
                nc.scalar.activation(
                    p_un[:, sl], s_ps[:], EXP,
                    bias=negmax_c[:, n:n + 1], scale=1.0,
                    accum_out=sum_c[:, n:n + 1])

            negM = stats.tile([128, 1], F32, tag="negM", name="negM")
            nc.vector.tensor_reduce(
                negM[:], negmax_c[:], axis=mybir.AxisListType.X,
                op=mybir.AluOpType.min)
            md_c = stats.tile([128, NE], F32, tag="md_c", name="md_c")
            nc.vector.tensor_scalar_sub(md_c[:], negmax_c[:], negM[:])
            alpha_c = stats.tile([128, NE], F32, tag="alpha_c", name="alpha_c")
            nc.scalar.activation(alpha_c[:], md_c[:], EXP, scale=-1.0)
            z_c = stats.tile([128, NE], F32, tag="z_c", name="z_c")
            nc.vector.tensor_mul(z_c[:], sum_c[:], alpha_c[:])
            zsum = stats.tile([128, 1], F32, tag="zsum", name="zsum")
            nc.vector.tensor_reduce(
                zsum[:], z_c[:], axis=mybir.AxisListType.X,
                op=mybir.AluOpType.add)
            rz = stats.tile([128, 1], F32, tag="rz", name="rz")
            nc.vector.reciprocal(rz[:], zsum[:])
            mult_c = stats.tile([128, NE], F32, tag="mult_c", name="mult_c")
            nc.vector.tensor_scalar_mul(mult_c[:], alpha_c[:], rz[:])
            for n in range(NE):
                sl = slice(n * 512, (n + 1) * 512)
                nc.vector.tensor_scalar_mul(
                    p_norm[:, sl], p_un[:, sl], mult_c[:, n:n + 1])
            nc.sync.dma_start(attn[b, m * 128:(m + 1) * 128, :], p_norm[:])
            return p_norm

        def emit_context(b, m, p_norm):
            pt = ptpool.tile([128, EB, 128], F32R, name="pt")
            for g in range(EB // 4):
                ptile = ps_tr.tile([128, 512], F32, tag="tr", name="ptile")
                for j in range(4):
                    e = g * 4 + j
                    nc.tensor.transpose(
                        ptile[:, j * 128:(j + 1) * 128],
                        p_norm[:, e * 128:(e + 1) * 128],
                        identity[:])
                nc.scalar.copy(
                    pt[:, g * 4:(g + 1) * 4, :].rearrange("p a b -> p (a b)"),
                    ptile[:])

            ctxt = cpool.tile([128, H], F32, name="ctxt")
            c_pss = [ps_c.tile([128, 512], F32, tag=f"ctx{hc}",
                               name=f"c_ps{hc}") for hc in range(NH)]
            for e in range(EB):
                for hc in range(NH):
                    nc.tensor.matmul(
                        c_pss[hc][:], pt[:, e, :],
                        e_nat[:, e, slice(hc * 512, (hc + 1) * 512)],
                        start=(e == 0), stop=(e == EB - 1))
            for hc in range(NH):
                nc.vector.tensor_copy(
                    ctxt[:, slice(hc * 512, (hc + 1) * 512)], c_pss[hc][:])
            nc.sync.dma_start(ctx_out[b, m * 128:(m + 1) * 128, :], ctxt[:])

        for rep in range(REPEAT):
            for b in range(B_PER_CORE):
                for e in range(EB):
                    nc.gpsimd.dma_start(
                        e_nat[:, e, :], enc[b, e * 128:(e + 1) * 128, :])

                bias_row = None
                if mask_enabled:
                    raw_row = rows.tile([1, ENC], F32, tag="raw")
                    bias_row = rows.tile([1, ENC], F32R, tag="bias")
                    nc.sync.dma_start(
                        raw_row[:],
                        enc[b, :, 0:1].rearrange("e one -> one e"))
                    nc.vector.tensor_scalar(
                        out=bias_row[:], in0=raw_row[:],
                        scalar1=0.0, scalar2=-1e30,
                        op0=mybir.AluOpType.is_equal,
                        op1=mybir.AluOpType.mult)

                for h in range(HB):
                    for eg in range(EB // 4):
                        ptile = ps_tr.tile([128, 512], F32, tag="tr",
                                           name="ptile")
                        for j in range(4):
                            e = eg * 4 + j
                            nc.tensor.transpose(
                                ptile[:, j * 128:(j + 1) * 128],
                                e_nat[:, e, h * 128:(h + 1) * 128]
                                .bitcast(F32),
                                identity[:])
                        nc.vector.tensor_copy(
                            kt[:, h, eg * 512:(eg + 1) * 512], ptile[:])

                pending = None
                for m in range(DB):
                    p_norm = emit_scores(b, m, bias_row)
                    if pending is not None:
                        emit_context(b, pending[0], pending[1])
                    pending = (m, p_norm)
                emit_context(b, pending[0], pending[1])

        if timing_mode:
            nc.sync.dma_start(dummy_out[:], dummy_in[:])

    return nc


# ---------------------------------------------------------------------------
# Reusable multi-core PJRT runner (axon path), same lowering as
# concourse.bass2jax.run_bass_via_pjrt but returning a callable that can be
# invoked repeatedly without re-jitting.
# ---------------------------------------------------------------------------
def make_runner(nc, n_cores):
    import jax
    from jax.sharding import Mesh, PartitionSpec
    from jax.experimental.shard_map import shard_map
    from concourse.bass2jax import (
        _bass_exec_p, install_neuronx_cc_hook, partition_id_tensor)

    install_neuronx_cc_hook()
    partition_name = (nc.partition_id_tensor.name
                      if nc.partition_id_tensor else None)

    in_names, out_names, out_avals, zero_outs = [], [], [], []
    for alloc in nc.m.functions[0].allocations:
        if not isinstance(alloc, mybir.MemoryLocationSet):
            continue
        name = alloc.memorylocations[0].name
        if alloc.kind == "ExternalInput":
            if name != partition_name:
                in_names.append(name)
        elif alloc.kind == "ExternalOutput":
            shape = tuple(alloc.tensor_shape)
            dtype = mybir.dt.np(alloc.dtype)
            out_names.append(name)
            out_avals.append(jax.core.ShapedArray(shape, dtype))
            zero_outs.append(np.zeros(shape, dtype))
    n_params = len(in_names)
    n_outs = len(out_avals)
    in_names_all = in_names + out_names
    if partition_name is not None:
        in_names_all = in_names_all + [partition_name]

    donate = tuple(range(n_params, n_params + n_outs))

    def _body(*args):
        operands = list(args)
        if partition_name is not None:
            operands.append(partition_id_tensor())
        outs = _bass_exec_p.bind(
            *operands,
            out_avals=tuple(out_avals),
            in_names=tuple(in_names_all),
            out_names=tuple(out_names),
            lowering_input_output_aliases=(),
            sim_require_finite=True,
            sim_require_nnan=True,
            nc=nc,
        )
        return tuple(outs)

    devices = jax.devices()[:n_cores]
    mesh = Mesh(np.asarray(devices), ("core",))
    in_specs = (PartitionSpec("core"),) * (n_params + n_outs)
    out_specs = (PartitionSpec("core"),) * n_outs
    sharded = jax.jit(
        shard_map(_body, mesh=mesh, in_specs=in_specs, out_specs=out_specs,
                  check_rep=False),
        donate_argnums=donate, keep_unused=True)

    def run(in_maps, async_mode=False):
        per_core = [[np.asarray(m[name]) for name in in_names]
                    for m in in_maps]
        concat_in = [
            np.concatenate([per_core[c][i] for c in range(n_cores)], axis=0)
            for i in range(n_params)
        ]
        concat_zero = [np.concatenate([z] * n_cores, axis=0)
                       for z in zero_outs]
        out_arrs = sharded(*concat_in, *concat_zero)
        if async_mode:
            return out_arrs
        out_arrs = [np.asarray(a) for a in out_arrs]
        results = []
        for c in range(n_cores):
            r = {}
            for i, name in enumerate(out_names):
                per = out_arrs[i].shape[0] // n_cores
                r[name] = out_arrs[i][c * per:(c + 1) * per]
            results.append(r)
        return results

    return run


_cached = {}


def _get_runner():
    if "run" not in _cached:
        nc = build_attention_nc()
        _cached["run"] = make_runner(nc, N_CORES)
    return _cached["run"]


def kernel(decoder_states: np.ndarray, encoder_states: np.ndarray):
    """Full-batch attention on 8 NeuronCores. Returns (context, attn)."""
    decoder_states = np.ascontiguousarray(decoder_states, dtype=np.float32)
    encoder_states = np.ascontiguousarray(encoder_states, dtype=np.float32)
    assert decoder_states.shape == (B, DEC, H), decoder_states.shape
    assert encoder_states.shape == (B, ENC, H), encoder_states.shape

    run = _get_runner()
    in_maps = [{"decoder_states": decoder_states[c * BPC:(c + 1) * BPC],
                "encoder_states": encoder_states[c * BPC:(c + 1) * BPC]}
               for c in range(N_CORES)]
    res = run(in_maps)
    context = np.concatenate([r["context"] for r in res], axis=0)
    attn = np.concatenate([r["attn"] for r in res], axis=0)
    return context, attn
